# revision 13
# baseline (speedup 1.0000x reference)
"""Trainium2 Bass kernel for nn_CrossAttentionBlock (B=8, C=256, H=W=48).

Sharding: data-parallel over batch B — one batch per NeuronCore (8 cores).

Per-core math (x: [C=256, N=2304] f32):
  LayerNorm folded into projection weights on host (W_eff = W * w_n,
  b_eff = b + W @ b_n, attention SCALE folded into Wq). The k-bias is
  dropped entirely (a per-query-column logit shift cancels in softmax);
  the v-bias is folded into the output-projection bias (bv contributes
  bv (x) rowsum to the unnormalized output, which normalizes to a
  constant): bp_eff = bp + Wp @ bv_eff.
  Stats (mean / mean-square) matmuls run in fp32r DIRECTLY on the raw
  f32 DMA'd activations (fp32r streams 1 cycle/row like bf16 for free
  dims >= 256) — no bf16 cast pass at all. x**2 for the mean-square is
  produced bf16 on Scalar (x2) / GpSimd (x1), both idle early.
  xn = (x - u) * rstd on DVE/GpSimd straight from f32, output bf16.
  Attention transposed: St[m,n] = sum_o k[o,m] q[o,n]; P = exp(St)
  (logits bounded, no row-max). Softmax denominator: P tiles are
  accumulated elementwise on Vector (bf16) into P_sum, then ONE
  ones-matmul per chunk broadcasts the column sum to all partitions
  (replaces 18 M=1 rowsum matmuls per chunk). 1/rowsum folded in
  AFTER the output projection (commutes with Wp).
  Software pipelining: query-chunk 0's attention m-tiles are merged
  into the x2 stats/k/vT production loop (each x2 chunk yields 4 more
  k/vT m-tiles, immediately consumed by chunk-0 S/exp/PV), so
  attention starts as soon as the first x2 chunk lands. Output
  projection + residual + DMA-out are fused per chunk.
"""

import os
import sys
import types
import ctypes
import contextlib

sys.path.insert(0, "/opt/trn_rl_repo")

import numpy as np
import ml_dtypes

# ---------------------------------------------------------------------------
# NTFF profile hook stub (antenv.axon_hooks is absent in this container; the
# ctypes shim mirrors trn_agent_boot). Only used when tracing is requested.
# ---------------------------------------------------------------------------


def _ntff_profile_via_ctypes(so_path):
    try:
        lib = ctypes.CDLL(so_path)
    except OSError:
        return None
    if not hasattr(lib, "axon_start_nrt_profile"):
        return None
    lib.axon_start_nrt_profile.argtypes = [
        ctypes.POINTER(ctypes.c_int64),
        ctypes.c_size_t,
    ]
    lib.axon_start_nrt_profile.restype = ctypes.c_int64
    lib.axon_stop_nrt_profile.argtypes = [ctypes.c_char_p]
    lib.axon_stop_nrt_profile.restype = ctypes.c_int64

    @contextlib.contextmanager
    def _hook(output_dir, device_ids):
        import jax

        jax.devices()
        if device_ids:
            ids = (ctypes.c_int64 * len(device_ids))(*device_ids)
            rc = lib.axon_start_nrt_profile(ids, len(device_ids))
        else:
            rc = lib.axon_start_nrt_profile(None, 0)
        if rc != 0:
            raise RuntimeError(f"axon_start_nrt_profile rc={rc}")
        try:
            yield
        finally:
            n = lib.axon_stop_nrt_profile(str(output_dir).encode())
            print(f"profile: {n} file(s) written to {output_dir}", file=sys.stderr)

    return _hook


if "antenv.axon_hooks" not in sys.modules:
    _hook = _ntff_profile_via_ctypes("/opt/axon/libaxon_pjrt.so")
    _mod = types.ModuleType("antenv.axon_hooks")
    _mod.get_axon_ntff_profile_hook = lambda: _hook
    sys.modules["antenv.axon_hooks"] = _mod

# ---------------------------------------------------------------------------

B, C, H, W = 8, 256, 48, 48
N = H * W  # 2304
SCALE = (C // 8) ** (-0.5)
EPS = 1e-6
CT = C // 128  # 2 channel tiles
MT = N // 128  # 18 m (key-token) tiles
CHUNKS = [(0, 512), (512, 512), (1024, 512), (1536, 512), (2048, 256)]

BF16 = ml_dtypes.bfloat16

_cache = {}
last_results = None  # BassKernelResults of the most recent run (for test.py)


def _build_program():
    import concourse.bacc as bacc
    import concourse.tile as tile
    import concourse.mybir as mybir
    from contextlib import ExitStack

    f32 = mybir.dt.float32
    f32r = mybir.dt.float32r
    bf16 = mybir.dt.bfloat16
    ADD = mybir.AluOpType.add
    SUB = mybir.AluOpType.subtract

    nc = bacc.Bacc("TRN2", target_bir_lowering=False, debug=False)

    x1_d = nc.dram_tensor("x1", [C, N], f32r, kind="ExternalInput").ap()
    x2_d = nc.dram_tensor("x2", [C, N], f32r, kind="ExternalInput").ap()
    wqt_d = nc.dram_tensor("wqt", [C, C], bf16, kind="ExternalInput").ap()
    wkt_d = nc.dram_tensor("wkt", [C, C], bf16, kind="ExternalInput").ap()
    wvt_d = nc.dram_tensor("wvt", [C, C], bf16, kind="ExternalInput").ap()
    wpt_d = nc.dram_tensor("wpt", [C, C], bf16, kind="ExternalInput").ap()
    # cvec columns: 0/1 = bq per o-tile, 4/5 = bp_eff per c-tile.
    cvec_d = nc.dram_tensor("cvec", [128, 6], f32, kind="ExternalInput").ap()
    # onesr: fp32r 1/C block, lhsT of the mean matmul on raw f32 x.
    onesr_d = nc.dram_tensor("onesr", [128, 128], f32r, kind="ExternalInput").ap()
    # cbf columns: 0:128 = 1/C (mean-square matmul lhsT), 132:260 = 1.0
    # (ones block, lhsT of the denominator colsum-broadcast matmul).
    cbf_d = nc.dram_tensor("cbf", [128, 260], bf16, kind="ExternalInput").ap()
    # bprow: effective output-projection bias row; onesw: K=1 ones rhs row.
    bprow_d = nc.dram_tensor("bprow", [1, C], bf16, kind="ExternalInput").ap()
    onesw_d = nc.dram_tensor("onesw", [1, 512], bf16, kind="ExternalInput").ap()
    out_d = nc.dram_tensor("out", [C, N], f32, kind="ExternalOutput").ap()

    with tile.TileContext(nc) as tc, ExitStack() as ctx:
        persist = ctx.enter_context(tc.tile_pool(name="persist", bufs=1))

        # ---- input + const DMA: chunk 0 first, weights interleaved -----
        x2_t = [
            persist.tile([128, N], f32r, tag=f"x2_{ct}", name=f"x2_{ct}")
            for ct in range(CT)
        ]
        x1_t = [
            persist.tile([128, N], f32r, tag=f"x1_{ct}", name=f"x1_{ct}")
            for ct in range(CT)
        ]

        def dma_chunk(x_t, x_d, ji):
            off, w = CHUNKS[ji]
            for ct in range(CT):
                nc.sync.dma_start(
                    x_t[ct][:, off : off + w],
                    x_d[ct * 128 : (ct + 1) * 128, off : off + w],
                )

        dma_chunk(x2_t, x2_d, 0)
        onesr = persist.tile([128, 128], f32r, tag="onesr", name="onesr")
        nc.sync.dma_start(onesr[:], onesr_d[:, :])
        cbf = persist.tile([128, 260], bf16, tag="cbf", name="cbf")
        nc.sync.dma_start(cbf[:], cbf_d[:, :])
        cvec = persist.tile([128, 6], f32, tag="cvec", name="cvec")
        nc.sync.dma_start(cvec[:], cvec_d[:, :])
        bprow = persist.tile([1, C], bf16, tag="bprow", name="bprow")
        nc.sync.dma_start(bprow[:], bprow_d[:, :])
        onesw = persist.tile([1, 512], bf16, tag="onesw", name="onesw")
        nc.sync.dma_start(onesw[:], onesw_d[:, :])
        dma_chunk(x1_t, x1_d, 0)

        w_tiles = {}
        wdefs = {nm: d for nm, d in (("k", wkt_d), ("v", wvt_d), ("q", wqt_d), ("p", wpt_d))}
        def dma_weight(nm):
            for ct in range(CT):
                t = persist.tile([128, C], bf16, tag=f"w{nm}{ct}", name=f"w{nm}{ct}")
                nc.sync.dma_start(t[:], wdefs[nm][ct * 128 : (ct + 1) * 128, :])
                w_tiles[(nm, ct)] = t

        dma_weight("k")
        dma_weight("v")
        dma_chunk(x2_t, x2_d, 1)
        dma_chunk(x1_t, x1_d, 1)
        dma_weight("q")
        dma_weight("p")
        for ji in range(2, len(CHUNKS)):
            dma_chunk(x2_t, x2_d, ji)
            dma_chunk(x1_t, x1_d, ji)
        x1_f = [t[:].bitcast(f32) for t in x1_t]
        x2_f = [t[:].bitcast(f32) for t in x2_t]

        # persistent intermediates
        k_t = [persist.tile([128, N], bf16, tag=f"k{ot}", name=f"k{ot}") for ot in range(CT)]
        vT_t = [persist.tile([128, C], bf16, tag=f"vT{m}", name=f"vT{m}") for m in range(MT)]
        xn1_t = [
            persist.tile([128, N], bf16, tag=f"xn1_{ct}", name=f"xn1_{ct}")
            for ct in range(CT)
        ]

        with (
            tc.tile_pool(name="mid2", bufs=1) as mid2,
            tc.tile_pool(name="scr", bufs=3) as scr,
            tc.tile_pool(name="qch", bufs=2) as qch,
            tc.tile_pool(name="pt", bufs=3) as pt_pool,
            tc.tile_pool(name="ascr", bufs=3) as ascr,
            tc.tile_pool(name="ps_a", bufs=1, space="PSUM") as ps_a,   # stats
            tc.tile_pool(name="ps_b", bufs=1, space="PSUM") as ps_b,   # k/q proj
            tc.tile_pool(name="ps_c", bufs=1, space="PSUM") as ps_c,   # vT proj
            tc.tile_pool(name="ps_d", bufs=1, space="PSUM") as ps_d,   # bc + outproj
            tc.tile_pool(name="ps_qk", bufs=2, space="PSUM") as ps_qk,
            tc.tile_pool(name="ps_o", bufs=2, space="PSUM") as ps_o,
        ):
            q_ch = {}

            def emit_x1_chunk(ji):
                # stats1 (squares on GpSimd) + xn1 + q projection for chunk ji
                off, w = CHUNKS[ji]
                ub = ps_a.tile([128, 512], f32, tag="sta", name="ub1")
                for ct in range(CT):
                    nc.tensor.matmul(
                        ub[:, :w],
                        onesr[:, :],
                        x1_t[ct][:, off : off + w],
                        start=(ct == 0),
                        stop=(ct == CT - 1),
                    )
                ms = ps_a.tile([128, 512], f32, tag="sta", name="ms1")
                for ct in range(CT):
                    xsq = ascr.tile([128, 512], bf16, tag="xsq1", name="xsq1")
                    nc.gpsimd.tensor_mul(
                        xsq[:, :w],
                        x1_f[ct][:, off : off + w],
                        x1_f[ct][:, off : off + w],
                    )
                    nc.tensor.matmul(
                        ms[:, :w],
                        cbf[:, 0:128],
                        xsq[:, :w],
                        start=(ct == 0),
                        stop=(ct == CT - 1),
                    )
                usq = ascr.tile([128, 512], f32, tag="usq1", name="usq1")
                nc.scalar.square(usq[:, :w], ub[:, :w])
                var = ascr.tile([128, 512], f32, tag="var1", name="var1")
                nc.vector.scalar_tensor_tensor(
                    var[:, :w], ms[:, :w], EPS, usq[:, :w], ADD, SUB
                )
                std = ascr.tile([128, 512], f32, tag="std1", name="std1")
                nc.scalar.activation(
                    std[:, :w], var[:, :w], mybir.ActivationFunctionType.Sqrt
                )
                rstd = ascr.tile([128, 512], f32, tag="rstd1", name="rstd1")
                nc.vector.reciprocal_approx_fast(rstd[:, :w], std[:, :w])
                for ct in range(CT):
                    d = ascr.tile([128, 512], f32, tag="xnd1", name="xnd1")
                    nc.vector.tensor_sub(
                        d[:, :w], x1_f[ct][:, off : off + w], ub[:, :w]
                    )
                    eng = nc.gpsimd if ct == 0 else nc.vector
                    eng.tensor_mul(
                        xn1_t[ct][:, off : off + w], d[:, :w], rstd[:, :w]
                    )
                # q projection
                for ot in range(CT):
                    ps = ps_b.tile([128, 512], f32, tag="pjq", name="qp")
                    for ct in range(CT):
                        nc.tensor.matmul(
                            ps[:, :w],
                            w_tiles[("q", ct)][:, ot * 128 : (ot + 1) * 128],
                            xn1_t[ct][:, off : off + w],
                            start=(ct == 0),
                            stop=(ct == CT - 1),
                        )
                    qt = qch.tile([128, 512], bf16, tag=f"q{ot}", name=f"q{ot}")
                    nc.vector.tensor_scalar_add(
                        qt[:, :w], ps[:, :w], cvec[:, 0 + ot : 1 + ot]
                    )
                    q_ch[(ji, ot)] = qt

            # ---- per-query-chunk attention state + emitters -------------
            class AttnState:
                pass

            def attn_begin(ji):
                s = AttnState()
                s.ji = ji
                s.off, s.w = CHUNKS[ji]
                s.st = {}
                s.o_ps = [
                    ps_o.tile([128, 512], f32, tag="o", name="o") for _ in range(CT)
                ]
                s.acc_v = ascr.tile([128, 512], bf16, tag="accv", name="accv")
                s.acc_g = ascr.tile([128, 512], bf16, tag="accg", name="accg")
                s.pt_hold = {}
                return s

            def attn_qk(s, m):
                ps = ps_qk.tile([128, 512], f32, tag="st", name="st")
                for ot in range(CT):
                    nc.tensor.matmul(
                        ps[:, : s.w],
                        k_t[ot][:, m * 128 : (m + 1) * 128],
                        q_ch[(s.ji, ot)][:, : s.w],
                        start=(ot == 0),
                        stop=(ot == CT - 1),
                    )
                s.st[m] = ps

            def attn_m(s, m):
                # exp + PV + denominator accumulate for m-tile m
                w = s.w
                pt = pt_pool.tile([128, 512], bf16, tag=f"pt{m%3}", name=f"pt{m%3}")
                nc.scalar.activation(
                    pt[:, :w], s.st[m][:, :w], mybir.ActivationFunctionType.Exp
                )
                del s.st[m]
                for c in range(CT):
                    nc.tensor.matmul(
                        s.o_ps[c][:, :w],
                        vT_t[m][:, c * 128 : (c + 1) * 128],
                        pt[:, :w],
                        start=(m == 0),
                        stop=(m == MT - 1),
                    )
                # dual denominator accumulators: GpSimd takes m%3!=1 (2/3),
                # Vector takes m%3==1 (1/3)
                if m < 2:
                    s.pt_hold[m] = pt
                elif m == 2:
                    nc.gpsimd.tensor_add(
                        s.acc_g[:, :w], s.pt_hold[0][:, :w], pt[:, :w]
                    )
                    del s.pt_hold[0]
                elif m == 3:
                    nc.vector.tensor_add(
                        s.acc_v[:, :w], s.pt_hold[1][:, :w], pt[:, :w]
                    )
                    del s.pt_hold[1]
                elif m % 3 == 1:
                    nc.vector.tensor_add(
                        s.acc_v[:, :w], s.acc_v[:, :w], pt[:, :w]
                    )
                else:
                    nc.gpsimd.tensor_add(
                        s.acc_g[:, :w], s.acc_g[:, :w], pt[:, :w]
                    )

            def attn_end(s):
                # denominator broadcast + output projection + residual + DMA
                w, off = s.w, s.off
                bc = ps_d.tile([128, 512], f32, tag="dd", name="bc")
                nc.tensor.matmul(
                    bc[:, :w], cbf[:, 132:260], s.acc_g[:, :w],
                    start=True, stop=False,
                )
                nc.tensor.matmul(
                    bc[:, :w], cbf[:, 132:260], s.acc_v[:, :w],
                    start=False, stop=True,
                )
                inv_b = ascr.tile([128, 512], f32, tag="invb", name="invb")
                nc.vector.reciprocal_approx_fast(inv_b[:, :w], bc[:, :w])
                # normalize BEFORE the Wp projection; bias via K=1 matmul
                ou = []
                for c in range(CT):
                    t = ascr.tile([128, 512], bf16, tag=f"ou{c}", name=f"ou{c}")
                    nc.vector.tensor_mul(t[:, :w], s.o_ps[c][:, :w], inv_b[:, :w])
                    ou.append(t)
                for ct in range(CT):
                    ps = ps_d.tile([128, 512], f32, tag="dd", name="pp")
                    for ci in range(CT):
                        nc.tensor.matmul(
                            ps[:, :w],
                            w_tiles[("p", ci)][:, ct * 128 : (ct + 1) * 128],
                            ou[ci][:, :w],
                            start=(ci == 0),
                            stop=False,
                        )
                    nc.tensor.matmul(
                        ps[:, :w],
                        bprow[0:1, ct * 128 : (ct + 1) * 128],
                        onesw[0:1, :w],
                        start=False,
                        stop=True,
                    )
                    ot_t = ascr.tile([128, 512], f32, tag=f"out{ct}", name=f"out{ct}")
                    nc.vector.tensor_add(
                        ot_t[:, :w], ps[:, :w], x1_f[ct][:, off : off + w]
                    )
                    nc.sync.dma_start(
                        out_d[ct * 128 : (ct + 1) * 128, off : off + w],
                        ot_t[:, :w],
                    )

            # ================= phase 1: x2 pipeline + merged chunk-0 ====
            emit_x1_chunk(0)
            s0 = attn_begin(0)
            for ji, (off, w) in enumerate(CHUNKS):
                # ---- x2 stats for chunk ji ----------------------------
                ub = ps_a.tile([128, 512], f32, tag="sta", name="ub")
                for ct in range(CT):
                    nc.tensor.matmul(
                        ub[:, :w],
                        onesr[:, :],
                        x2_t[ct][:, off : off + w],
                        start=(ct == 0),
                        stop=(ct == CT - 1),
                    )
                ms = ps_a.tile([128, 512], f32, tag="sta", name="ms")
                for ct in range(CT):
                    xsq = scr.tile([128, 512], bf16, tag="xsqc", name="xsqc")
                    nc.scalar.square(xsq[:, :w], x2_f[ct][:, off : off + w])
                    nc.tensor.matmul(
                        ms[:, :w],
                        cbf[:, 0:128],
                        xsq[:, :w],
                        start=(ct == 0),
                        stop=(ct == CT - 1),
                    )
                usq = scr.tile([128, 512], f32, tag="usq", name="usq")
                nc.scalar.square(usq[:, :w], ub[:, :w])
                var = scr.tile([128, 512], f32, tag="var", name="var")
                nc.vector.scalar_tensor_tensor(
                    var[:, :w], ms[:, :w], EPS, usq[:, :w], ADD, SUB
                )
                std = scr.tile([128, 512], f32, tag="std", name="std")
                nc.scalar.activation(
                    std[:, :w], var[:, :w], mybir.ActivationFunctionType.Sqrt
                )
                rstd = scr.tile([128, 512], f32, tag="rstd", name="rstd")
                nc.vector.reciprocal_approx_fast(rstd[:, :w], std[:, :w])
                xn2 = {}
                for ct in range(CT):
                    d = scr.tile([128, 512], f32, tag="xnd", name="xnd")
                    nc.vector.tensor_sub(
                        d[:, :w], x2_f[ct][:, off : off + w], ub[:, :w]
                    )
                    xt = mid2.tile([128, 512], bf16, tag=f"xn2_{ct}{ji}", name=f"xn2_{ct}{ji}")
                    eng = nc.gpsimd if ct == 0 else nc.vector
                    eng.tensor_mul(xt[:, :w], d[:, :w], rstd[:, :w])
                    xn2[ct] = xt

                # ---- k projection for chunk ji (k-bias dropped) --------
                for ot in range(CT):
                    ps = ps_b.tile([128, 512], f32, tag="pjq", name="pj")
                    for ct in range(CT):
                        nc.tensor.matmul(
                            ps[:, :w],
                            w_tiles[("k", ct)][:, ot * 128 : (ot + 1) * 128],
                            xn2[ct][:, :w],
                            start=(ct == 0),
                            stop=(ct == CT - 1),
                        )
                    nc.vector.tensor_copy(k_t[ot][:, off : off + w], ps[:, :w])

                # ---- vT + merged chunk-0 attention m-tiles -------------
                for m in range(off // 128, (off + w) // 128):
                    coff = m * 128 - off
                    ps = ps_c.tile([128, C], f32, tag="pv", name="pv")
                    for ct in range(CT):
                        nc.tensor.matmul(
                            ps[:],
                            xn2[ct][:, coff : coff + 128],
                            w_tiles[("v", ct)][:, :],
                            start=(ct == 0),
                            stop=(ct == CT - 1),
                        )
                    nc.vector.tensor_copy(vT_t[m][:], ps[:])
                    # chunk-0 attention for this m-tile
                    attn_qk(s0, m)
                    attn_m(s0, m)
                if ji == 2:
                    emit_x1_chunk(1)
            attn_end(s0)

            # ================= chunks 1..4: pure attention ===============
            for ji in range(1, len(CHUNKS)):
                s = attn_begin(ji)
                attn_qk(s, 0)
                for m in range(MT):
                    if m + 1 < MT:
                        attn_qk(s, m + 1)
                    if m == 8 and ji + 1 < len(CHUNKS):
                        emit_x1_chunk(ji + 1)
                    attn_m(s, m)
                attn_end(s)

    nc.compile()
    return nc


def _host_prep(inputs):
    f = lambda k: np.asarray(inputs[k], dtype=np.float32)
    Wq, Wk, Wv, Wp = f("Wq"), f("Wk"), f("Wv"), f("Wp")
    bq, bk, bv, bp = f("bq"), f("bk"), f("bv"), f("bp")
    w_nq, b_nq, w_nkv, b_nkv = f("w_nq"), f("b_nq"), f("w_nkv"), f("b_nkv")

    Wq_eff = Wq * w_nq[None, :] * SCALE
    bq_eff = SCALE * (bq + Wq @ b_nq)
    Wk_eff = Wk * w_nkv[None, :]
    Wv_eff = Wv * w_nkv[None, :]
    bv_eff = bv + Wv @ b_nkv
    # v-bias folded into the output-projection bias (see module docstring)
    bp_eff = bp + Wp @ bv_eff

    wqt = np.ascontiguousarray(Wq_eff.T).astype(BF16)
    wkt = np.ascontiguousarray(Wk_eff.T).astype(BF16)
    wvt = np.ascontiguousarray(Wv_eff.T).astype(BF16)
    wpt = np.ascontiguousarray(Wp.T).astype(BF16)

    cvec = np.zeros((128, 6), np.float32)
    cvec[:, 0] = bq_eff[0:128]
    cvec[:, 1] = bq_eff[128:256]

    onesr = np.full((128, 128), 1.0 / C, np.float32)

    cbf = np.zeros((128, 260), np.float32)
    cbf[:, 0:128] = 1.0 / C
    cbf[:, 132:260] = 1.0
    cbf = cbf.astype(BF16)

    bprow = bp_eff.reshape(1, C).astype(BF16)
    onesw = np.ones((1, 512), np.float32).astype(BF16)
    return dict(
        wqt=wqt, wkt=wkt, wvt=wvt, wpt=wpt, cvec=cvec, onesr=onesr, cbf=cbf,
        bprow=bprow, onesw=onesw,
    )


def _maybe_patch_ldw_opt():
    if os.environ.get("BASS_LDW_OPT", "0") != "1":
        return
    import concourse.bass_utils as bu
    if getattr(bu, "_ldw_patch", False):
        return
    orig = bu.run_command
    def patched(argv, **kw):
        if isinstance(argv, list):
            argv = [a.replace("--enable-ldw-opt=false", "--enable-ldw-opt=true") for a in argv]
        return orig(argv, **kw)
    bu.run_command = patched
    bu._ldw_patch = True


def kernel(**inputs):
    global last_results
    _maybe_patch_ldw_opt()
    from concourse.bass_utils import run_bass_kernel_spmd

    if "nc" not in _cache:
        _cache["nc"] = _build_program()
    nc = _cache["nc"]

    shared = _host_prep(inputs)
    x1 = np.asarray(inputs["x1"], dtype=np.float32).reshape(B, C, N)
    x2 = np.asarray(inputs["x2"], dtype=np.float32).reshape(B, C, N)

    in_maps = []
    for b in range(B):
        m = dict(shared)
        m["x1"] = np.ascontiguousarray(x1[b])
        m["x2"] = np.ascontiguousarray(x2[b])
        in_maps.append(m)

    trace = os.environ.get("BASS_KERNEL_TRACE", "0") == "1"
    res = run_bass_kernel_spmd(
        nc, in_maps, core_ids=list(range(B)), trace=trace
    )
    last_results = res
    out = np.stack([res.results[b]["out"].reshape(C, H, W) for b in range(B)])
    return out.astype(np.float32)
